# revision 14
# baseline (speedup 1.0000x reference)
"""Trainium2 Bass kernel for nn_GAT_NFM (2x GAT encoder layers + NFM bilinear
pooling + projection) on 8 NeuronCores.

Sharding: nodes are partitioned contiguously across the 8 cores (N/8 each);
edges are partitioned by src node (the segment/aggregation axis). Each core
computes its shard of the per-layer transformed features, the shards are
all-gathered into a full per-core bf16 feature table in HBM, and each core
then gathers its edges' dst rows from that table (dma_gather), computes the
per-edge attention weight w = exp(sigmoid(edge_val * (f1[src] + f2[dst])))
(the segment-max in the reference cancels algebraically in the softmax), and
scatter-adds w-weighted dst rows per src node with a w-weighted one-hot
matmul on the TensorEngine: out[i] = (sum_e w_e*Hw[dst_e]) / (sum_e w_e).

Feature table row (bf16): [Hw (Dl) | f2 | 1.0 | zero pad to 256B multiple];
the ones column folds the softmax denominator into the same matmul.  f1 is
fetched per-edge from a core-LOCAL scalar table [NSH, 128] bf16 (row i =
[f1_i, 0...]) gathered by core-local src index (fits int16, no all-gather).
dma_gather needs int16 indices, so the full table is split into <=32k-row
buckets and edges are grouped (core, node-tile, dst-bucket); groups are
padded to 128 edges with idx-0/srel=-1 dummies so every core runs the same
static program.  Weight matrices are augmented on the host:
W_ext = [W | W @ v_b | W @ v_a] so Hw, f2, f1 come from one matmul.
"""

import math
import os

import numpy as np

import concourse.bass as bass
import concourse.bacc as bacc
import concourse.mybir as mybir
import concourse.tile as tile
from concourse.bass_utils import run_bass_kernel_spmd
from concourse.masks import make_identity

P = 128
N_CORES = 8
F32 = mybir.dt.float32
BF16 = mybir.dt.bfloat16
I32 = mybir.dt.int32
I16 = mybir.dt.int16
AF = mybir.ActivationFunctionType
OP = mybir.AluOpType


# ----------------------------------------------------------------- host prep

def _prep(inputs, n_cores=N_CORES, bucket_cap=25000):
    x = np.ascontiguousarray(np.asarray(inputs["x"], dtype=np.float32))
    ev = np.asarray(inputs["edge_val"], dtype=np.float32)
    src = np.asarray(inputs["edge_src"], dtype=np.int64)
    dst = np.asarray(inputs["edge_dst"], dtype=np.int64)
    W0 = np.asarray(inputs["W0"], dtype=np.float32)
    W1 = np.asarray(inputs["W1"], dtype=np.float32)
    v00 = np.asarray(inputs["v0_0"], dtype=np.float32)
    v01 = np.asarray(inputs["v0_1"], dtype=np.float32)
    v10 = np.asarray(inputs["v1_0"], dtype=np.float32)
    v11 = np.asarray(inputs["v1_1"], dtype=np.float32)
    fme = np.asarray(inputs["fm_emb"], dtype=np.float32)
    pjw = np.asarray(inputs["proj_W"], dtype=np.float32)
    pjb = np.asarray(inputs["proj_b"], dtype=np.float32)

    N, Din = x.shape
    E = src.shape[0]
    D0 = W0.shape[1]          # 256
    D1 = W1.shape[1]          # 128
    FM = fme.shape[1]         # 64
    NCLS = pjw.shape[1]       # 64
    assert N % n_cores == 0
    NSH = N // n_cores
    NT = math.ceil(NSH / P)
    assert NSH < (1 << 15), "local scalar-gather index must fit int16"

    # bf16 feature-table row widths: multiple of 128 bf16 (=256B)
    C0 = ((D0 + 2 + 127) // 128) * 128     # 384 for D0=256
    C1 = ((D1 + 2 + 127) // 128) * 128     # 256 for D1=128
    CS = 128                               # local scalar-table row (256B)

    # dst buckets (int16 index range)
    NB = max(1, math.ceil(N / min(bucket_cap, 32000)))
    BSZ = math.ceil(N / NB)

    # ---- edge grouping: (core, node-tile of src, dst-bucket)
    loc = src % NSH
    core_of = src // NSH
    ltile = loc // P
    buck = dst // BSZ
    key = (core_of * NT + ltile) * NB + buck
    order = np.argsort(key, kind="stable")
    sdst = dst[order]
    sev = ev[order]
    skey = key[order]
    sloc = loc[order]

    cnt = np.bincount(skey, minlength=n_cores * NT * NB)
    cnt = cnt.reshape(n_cores, NT, NB)
    SZ = np.maximum(P, ((cnt.max(axis=0) + P - 1) // P) * P)   # [NT, NB] slots
    TPT = (SZ.sum(axis=1) // P).astype(np.int64)               # [NT] tiles
    CUM = np.zeros(NT + 1, np.int64)
    CUM[1:] = np.cumsum(TPT)
    TOT = int(CUM[-1])                                         # tiles per core
    TOTS = TOT * P                                             # slots per core
    # slot offset of each (nt, b) group
    OFF = np.zeros((NT, NB), np.int64)
    run = 0
    for nt in range(NT):
        for b in range(NB):
            OFF[nt, b] = run
            run += SZ[nt, b]
    assert run == TOTS

    grp = np.zeros(n_cores * NT * NB + 1, np.int64)
    grp[1:] = np.cumsum(cnt.reshape(-1))
    within = np.arange(E, dtype=np.int64) - grp[skey]
    snt = (skey // NB) % NT
    sb = skey % NB
    pad_pos = OFF[snt, sb] + within

    dst16 = np.zeros((n_cores, TOTS), np.int16)
    src16 = np.zeros((n_cores, TOTS), np.int16)
    srel = np.full((n_cores, TOTS), -1.0, np.float32)
    aval = np.zeros((n_cores, TOTS), np.float32)
    ci = core_of[order]
    dst16[ci, pad_pos] = (sdst - sb * BSZ).astype(np.int16)
    src16[ci, pad_pos] = sloc.astype(np.int16)
    srel[ci, pad_pos] = (sloc % P).astype(np.float32)
    aval[ci, pad_pos] = sev

    def to_cols(a):        # [TOTS] slot-major -> [P, TOT] (slot = col*128+p)
        return np.ascontiguousarray(a.reshape(TOT, P).T)

    def to_wrap16(a):      # [TOTS] -> [128, TOTS//16] 16-wrapped + replicated
        w = np.ascontiguousarray(a.reshape(TOTS // 16, 16).T)   # [16, TOTS/16]
        return np.ascontiguousarray(np.tile(w, (8, 1)))

    # host-side tiny weight prep (replicated across cores)
    w0e = np.ascontiguousarray(
        np.concatenate([W0, W0 @ v01, W0 @ v00], axis=1))          # [Din, D0+2]
    w1e = np.ascontiguousarray(
        np.concatenate([W1, W1 @ v11, W1 @ v10], axis=1))          # [D0, D1+2]
    ee2 = np.ascontiguousarray(
        np.concatenate([fme, fme * fme], axis=1))                  # [Din, 2FM]
    pja = np.ascontiguousarray(pjw[:D1])                           # [D1, NCLS]
    pjbm = np.ascontiguousarray(0.5 * pjw[D1:])                    # [FM, NCLS]
    pbias = np.ascontiguousarray(pjb[None, :])                     # [1, NCLS]
    iota = np.broadcast_to(np.arange(P, dtype=np.float32), (P, P)).copy()

    in_maps = []
    for c in range(n_cores):
        xt = np.ascontiguousarray(x[c * NSH:(c + 1) * NSH].T)      # [Din, NSH]
        in_maps.append({
            "xt": xt,
            "idxf": to_wrap16(dst16[c]),
            "idxs": to_wrap16(src16[c]),
            "srel": to_cols(srel[c]),
            "aval": to_cols(aval[c]),
            "w0e": w0e, "w1e": w1e, "ee2": ee2,
            "pja": pja, "pjb": pjbm, "pbias": pbias, "iota": iota,
        })

    cfg = dict(N=N, E=E, Din=Din, D0=D0, D1=D1, FM=FM, NCLS=NCLS,
               NSH=NSH, NT=NT, NB=NB, BSZ=BSZ,
               SZ=[[int(v) for v in row] for row in SZ],
               OFF=[[int(v) for v in row] for row in OFF],
               TPT=[int(t) for t in TPT], CUM=[int(c) for c in CUM],
               TOT=TOT, C0=C0, C1=C1, CS=CS, n_cores=n_cores)
    return cfg, in_maps


# ------------------------------------------------------------ device program

def _build(cfg):
    N = cfg["N"]; Din = cfg["Din"]; D0 = cfg["D0"]; D1 = cfg["D1"]
    FM = cfg["FM"]; NCLS = cfg["NCLS"]; NSH = cfg["NSH"]; NT = cfg["NT"]
    NB = cfg["NB"]; BSZ = cfg["BSZ"]; SZ = cfg["SZ"]; OFF = cfg["OFF"]
    TPT = cfg["TPT"]; CUM = cfg["CUM"]; TOT = cfg["TOT"]
    C0 = cfg["C0"]; C1 = cfg["C1"]; CS = cfg["CS"]; n_cores = cfg["n_cores"]
    TPTmax = max(TPT)
    KD = Din // P
    KD0 = D0 // P
    FM2 = 2 * FM

    nc = bacc.Bacc("TRN2", target_bir_lowering=False, debug=False,
                   num_devices=n_cores)

    xt_d = nc.dram_tensor("xt", [Din, NSH], F32, kind="ExternalInput")
    idxf_d = nc.dram_tensor("idxf", [P, TOT * 8], I16, kind="ExternalInput")
    idxs_d = nc.dram_tensor("idxs", [P, TOT * 8], I16, kind="ExternalInput")
    srel_d = nc.dram_tensor("srel", [P, TOT], F32, kind="ExternalInput")
    aval_d = nc.dram_tensor("aval", [P, TOT], F32, kind="ExternalInput")
    w0e_d = nc.dram_tensor("w0e", [Din, D0 + 2], F32, kind="ExternalInput")
    w1e_d = nc.dram_tensor("w1e", [D0, D1 + 2], F32, kind="ExternalInput")
    ee2_d = nc.dram_tensor("ee2", [Din, FM2], F32, kind="ExternalInput")
    pja_d = nc.dram_tensor("pja", [D1, NCLS], F32, kind="ExternalInput")
    pjb_d = nc.dram_tensor("pjb", [FM, NCLS], F32, kind="ExternalInput")
    pbias_d = nc.dram_tensor("pbias", [1, NCLS], F32, kind="ExternalInput")
    iota_d = nc.dram_tensor("iota", [P, P], F32, kind="ExternalInput")
    out_d = nc.dram_tensor("out", [NSH, NCLS], F32, kind="ExternalOutput")

    def tw(nt):
        return min(P, NSH - nt * P)

    stage = int(os.environ.get("KSTAGE", "9"))
    kedge = int(os.environ.get("KEDGE", "9"))

    with tile.TileContext(nc) as tc:
        with tc.tile_pool(name="dram", bufs=1, space="DRAM") as dram, \
             tc.tile_pool(name="const", bufs=1) as cpool, \
             tc.tile_pool(name="meta", bufs=1) as mpool:

            T0L = dram.tile([NSH, C0], BF16)
            T0F = dram.tile([N, C0], BF16)
            T0S = dram.tile([NSH, CS], BF16)
            T1L = dram.tile([NSH, C1], BF16)
            T1F = dram.tile([N, C1], BF16)
            T1S = dram.tile([NSH, CS], BF16)
            H1T = dram.tile([D0, NSH], F32)
            NFMT = dram.tile([FM, NSH], F32)

            # constants
            iota_t = cpool.tile([P, P], F32)
            nc.sync.dma_start(out=iota_t[:], in_=iota_d[:, :])
            ident = cpool.tile([P, P], F32)
            make_identity(nc, ident[:])
            ones_row = cpool.tile([1, P], F32)
            nc.vector.memset(ones_row[:], 1.0)
            w0e_t = [cpool.tile([P, D0 + 2], F32, tag=f"w0e{k}", name=f"w0e{k}")
                     for k in range(KD)]
            for k in range(KD):
                nc.sync.dma_start(out=w0e_t[k][:], in_=w0e_d[k * P:(k + 1) * P, :])
            w1e_t = [cpool.tile([P, D1 + 2], F32, tag=f"w1e{k}", name=f"w1e{k}")
                     for k in range(KD0)]
            for k in range(KD0):
                nc.sync.dma_start(out=w1e_t[k][:], in_=w1e_d[k * P:(k + 1) * P, :])
            ee2_t = [cpool.tile([P, FM2], F32, tag=f"ee2{k}", name=f"ee2{k}")
                     for k in range(KD)]
            for k in range(KD):
                nc.sync.dma_start(out=ee2_t[k][:], in_=ee2_d[k * P:(k + 1) * P, :])
            pja_t = cpool.tile([D1, NCLS], F32)
            nc.sync.dma_start(out=pja_t[:], in_=pja_d[:, :])
            pjb_t = cpool.tile([FM, NCLS], F32)
            nc.sync.dma_start(out=pjb_t[:], in_=pjb_d[:, :])
            pbias_t = cpool.tile([1, NCLS], F32)
            nc.sync.dma_start(out=pbias_t[:], in_=pbias_d[:, :])

            # edge metadata resident across both edge phases
            srel_t = mpool.tile([P, TOT], F32)
            aval_t = mpool.tile([P, TOT], F32)
            nc.sync.dma_start(out=srel_t[:], in_=srel_d[:, :])
            nc.sync.dma_start(out=aval_t[:], in_=aval_d[:, :])

            # ---------------- phase A: Hw0|f2|f1 = x @ w0e ; nfmT = f(x@ee2)
            with tc.tile_pool(name="a_sb", bufs=3) as asb, \
                 tc.tile_pool(name="a_xt", bufs=8) as axt, \
                 tc.tile_pool(name="a_ps", bufs=2, space="PSUM") as aps, \
                 tc.tile_pool(name="a_nf", bufs=2, space="PSUM") as anf:
                for jc in range(0, NT, 4):
                    tiles = list(range(jc, min(jc + 4, NT)))
                    n0 = jc * P
                    cw = sum(tw(t) for t in tiles)
                    xts = []
                    for k in range(KD):
                        xt = axt.tile([P, 4 * P], F32, tag="xt")
                        nc.sync.dma_start(out=xt[:, :cw],
                                          in_=xt_d[k * P:(k + 1) * P, n0:n0 + cw])
                        xts.append(xt)
                    nf1 = anf.tile([FM, 4 * P], F32, tag="nf1", space="PSUM")
                    nf2 = anf.tile([FM, 4 * P], F32, tag="nf2", space="PSUM")
                    for k in range(KD):
                        nc.tensor.matmul(out=nf1[:, :cw], lhsT=ee2_t[k][:, :FM],
                                         rhs=xts[k][:, :cw],
                                         start=(k == 0), stop=(k == KD - 1))
                    for k in range(KD):
                        nc.tensor.matmul(out=nf2[:, :cw],
                                         lhsT=ee2_t[k][:, FM:FM2],
                                         rhs=xts[k][:, :cw],
                                         start=(k == 0), stop=(k == KD - 1))
                    s1 = asb.tile([FM, 4 * P], F32, tag="nfs1")
                    nc.vector.tensor_copy(out=s1[:, :cw], in_=nf1[:, :cw])
                    s2 = asb.tile([FM, 4 * P], F32, tag="nfs2")
                    nc.vector.tensor_copy(out=s2[:, :cw], in_=nf2[:, :cw])
                    nfo = asb.tile([FM, 4 * P], F32, tag="nfo")
                    nc.vector.tensor_tensor(out=nfo[:, :cw], in0=s1[:, :cw],
                                            in1=s1[:, :cw], op=OP.mult)
                    nc.vector.tensor_tensor(out=nfo[:, :cw], in0=nfo[:, :cw],
                                            in1=s2[:, :cw], op=OP.subtract)
                    nc.sync.dma_start(out=NFMT[:, n0:n0 + cw], in_=nfo[:, :cw])
                    off = 0
                    for t in tiles:
                        wm = tw(t)
                        ap_ = aps.tile([P, D0 + 2], F32, tag="aps", space="PSUM")
                        for k in range(KD):
                            nc.tensor.matmul(out=ap_[:wm, :],
                                             lhsT=xts[k][:, off:off + wm],
                                             rhs=w0e_t[k][:],
                                             start=(k == 0), stop=(k == KD - 1))
                        st = asb.tile([P, C0], BF16, tag="st")
                        nc.vector.tensor_copy(out=st[:wm, 0:D0 + 1],
                                              in_=ap_[:wm, 0:D0 + 1])
                        nc.vector.memset(st[:, D0 + 1:D0 + 2], 1.0)
                        nc.vector.memset(st[:, D0 + 2:C0], 0.0)
                        nc.gpsimd.dma_start(out=T0L[t * P:t * P + wm, :],
                                            in_=st[:wm, :])
                        sc = asb.tile([P, CS], BF16, tag="sc")
                        nc.vector.memset(sc[:, :], 0.0)
                        nc.vector.tensor_copy(out=sc[:wm, 0:1],
                                              in_=ap_[:wm, D0 + 1:D0 + 2])
                        nc.sync.dma_start(out=T0S[t * P:t * P + wm, :],
                                          in_=sc[:wm, :])
                        off += wm

            # ---------------- all-gather T0
            if stage >= 2:
             nc.gpsimd.collective_compute(
                "AllGather", OP.bypass,
                replica_groups=[list(range(n_cores))],
                ins=[T0L[:].opt()], outs=[T0F[:].opt()])

            # ---------------- edge aggregation (shared by both layers)
            def edge_phase(TF, TS, CC, DD, h_out):
                with tc.tile_pool(name="e_g", bufs=2) as gp, \
                     tc.tile_pool(name="e_gs", bufs=2) as gsp, \
                     tc.tile_pool(name="e_ix", bufs=3) as ixp, \
                     tc.tile_pool(name="e_w", bufs=2) as wp, \
                     tc.tile_pool(name="e_oh", bufs=4) as ohp, \
                     tc.tile_pool(name="e_ps", bufs=2, space="PSUM") as psp, \
                     tc.tile_pool(name="e_tp", bufs=2, space="PSUM") as tpp, \
                     tc.tile_pool(name="e_sb", bufs=3) as esb:
                    for nt in range(NT):
                        tpt = TPT[nt]
                        c0 = CUM[nt]
                        wm = tw(nt)
                        # gathered feature rows for all this tile's edges
                        g = gp.tile([P, TPTmax, CC], BF16, tag="g")
                        ixf = ixp.tile([P, TPTmax * 8], I16, tag="ixf")
                        nc.sync.dma_start(
                            out=ixf[:, :tpt * 8],
                            in_=idxf_d[:, c0 * 8:(c0 + tpt) * 8])
                        for b in range(NB):
                            bn = min(BSZ, N - b * BSZ)
                            for z0 in range(0, SZ[nt][b], 2048):
                                sz = min(2048, SZ[nt][b] - z0)
                                o8 = (OFF[nt][b] - CUM[nt] * P + z0) // 16
                                ot = (OFF[nt][b] - CUM[nt] * P + z0) // P
                                nc.gpsimd.dma_gather(
                                    out_ap=g[:, ot:ot + sz // P, :],
                                    in_ap=TF[b * BSZ:b * BSZ + bn, :],
                                    idxs_ap=ixf[:, o8:o8 + sz // 16],
                                    num_idxs=sz, num_idxs_reg=sz,
                                    elem_size=CC, single_packet=False)
                        # per-edge f1[src] from the local scalar table
                        gs = gsp.tile([P, TPTmax, CS], BF16, tag="gs")
                        ixs = ixp.tile([P, TPTmax * 8], I16, tag="ixs")
                        nc.sync.dma_start(
                            out=ixs[:, :tpt * 8],
                            in_=idxs_d[:, c0 * 8:(c0 + tpt) * 8])
                        for q0 in range(0, tpt, 16):
                            qn = min(16, tpt - q0)
                            nc.gpsimd.dma_gather(
                                out_ap=gs[:, q0:q0 + qn, :], in_ap=TS[:, :],
                                idxs_ap=ixs[:, q0 * 8:(q0 + qn) * 8],
                                num_idxs=qn * P, num_idxs_reg=qn * P,
                                elem_size=CS, single_packet=False)
                        if kedge <= 1:
                            nc.gpsimd.dma_start(
                                out=H1T[0:P, nt * P:nt * P + wm],
                                in_=g[:wm, 0, 0:P].rearrange("a b -> b a") if False else g[:, 0, 0:wm])
                            continue
                        # w = exp(sigmoid(aval * (f1 + f2)))
                        w = wp.tile([P, TPTmax], F32, tag="w")
                        nc.vector.tensor_tensor(out=w[:, :tpt],
                                                in0=gs[:, :tpt, 0],
                                                in1=g[:, :tpt, DD], op=OP.add)
                        nc.vector.tensor_tensor(out=w[:, :tpt], in0=w[:, :tpt],
                                                in1=aval_t[:, c0:c0 + tpt],
                                                op=OP.mult)
                        nc.scalar.activation(w[:, :tpt], w[:, :tpt], AF.Sigmoid)
                        nc.scalar.activation(w[:, :tpt], w[:, :tpt], AF.Exp)
                        if kedge <= 2:
                            nc.sync.dma_start(
                                out=H1T[0:P, nt * P:nt * P + wm],
                                in_=w[:, 0:1].to_broadcast([P, wm]))
                            continue
                        # accumulate [num | den] with w-weighted one-hots
                        ps = psp.tile([P, DD + 2], F32, tag="ps", space="PSUM")
                        for t in range(tpt):
                            oh = ohp.tile([P, P], BF16, tag="oh")
                            nc.vector.tensor_scalar(
                                out=oh[:], in0=iota_t[:],
                                scalar1=srel_t[:, c0 + t:c0 + t + 1],
                                scalar2=w[:, t:t + 1],
                                op0=OP.is_equal, op1=OP.mult)
                            nc.tensor.matmul(out=ps[:], lhsT=oh[:],
                                             rhs=g[:, t, 0:DD + 2],
                                             start=(t == 0), stop=(t == tpt - 1))
                        den = esb.tile([P, 1], F32, tag="den")
                        nc.vector.tensor_scalar(out=den[:], in0=ps[:, DD + 1:DD + 2],
                                                scalar1=1e-30, scalar2=None,
                                                op0=OP.add)
                        rec = esb.tile([P, 1], F32, tag="rec")
                        nc.vector.reciprocal(rec[:], den[:])
                        hsb = esb.tile([P, DD], F32, tag="hsb")
                        nc.vector.tensor_scalar(out=hsb[:], in0=ps[:, 0:DD],
                                                scalar1=rec[:, :1], scalar2=None,
                                                op0=OP.mult)
                        if kedge <= 3:
                            continue
                        h_out(nt, wm, hsb, (tpp, esb))

            def h1_out(nt, wm, hsb, pools):
                tpp, esb = pools
                for b in range(D0 // P):
                    tp = tpp.tile([P, P], F32, tag="tp", space="PSUM")
                    nc.tensor.transpose(out=tp[:, :wm],
                                        in_=hsb[:wm, b * P:(b + 1) * P],
                                        identity=ident[:wm, :wm])
                    ht = esb.tile([P, P], F32, tag="ht")
                    nc.vector.tensor_copy(out=ht[:, :wm], in_=tp[:, :wm])
                    nc.sync.dma_start(
                        out=H1T[b * P:(b + 1) * P, nt * P:nt * P + wm],
                        in_=ht[:, :wm])

            if stage >= 3:
             edge_phase(T0F, T0S, C0, D0, h1_out)

            # ---------------- phase B': T1 = [H1 @ w1e | 1 | f1'] ; all-gather
            with tc.tile_pool(name="b_sb", bufs=3) as bsb, \
                 tc.tile_pool(name="b_ps", bufs=2, space="PSUM") as bps:
                for nt in range(NT if stage >= 4 else 0):
                    wm = tw(nt)
                    n0 = nt * P
                    bp = bps.tile([P, D1 + 2], F32, tag="bp", space="PSUM")
                    for k in range(KD0):
                        ht = bsb.tile([P, P], F32, tag="htin")
                        nc.sync.dma_start(out=ht[:, :wm],
                                          in_=H1T[k * P:(k + 1) * P, n0:n0 + wm])
                        nc.tensor.matmul(out=bp[:wm, :], lhsT=ht[:, :wm],
                                         rhs=w1e_t[k][:],
                                         start=(k == 0), stop=(k == KD0 - 1))
                    st = bsb.tile([P, C1], BF16, tag="st2")
                    nc.vector.tensor_copy(out=st[:wm, 0:D1 + 1],
                                          in_=bp[:wm, 0:D1 + 1])
                    nc.vector.memset(st[:, D1 + 1:D1 + 2], 1.0)
                    nc.vector.memset(st[:, D1 + 2:C1], 0.0)
                    nc.gpsimd.dma_start(out=T1L[n0:n0 + wm, :],
                                        in_=st[:wm, :])
                    sc = bsb.tile([P, CS], BF16, tag="sc2")
                    nc.vector.memset(sc[:, :], 0.0)
                    nc.vector.tensor_copy(out=sc[:wm, 0:1],
                                          in_=bp[:wm, D1 + 1:D1 + 2])
                    nc.sync.dma_start(out=T1S[n0:n0 + wm, :], in_=sc[:wm, :])

            if stage >= 5:
             nc.gpsimd.collective_compute(
                "AllGather", OP.bypass,
                replica_groups=[list(range(n_cores))],
                ins=[T1L[:].opt()], outs=[T1F[:].opt()])

            # ---------------- phase C: layer-2 aggregation + final projection
            with tc.tile_pool(name="c_fp", bufs=2, space="PSUM") as cfp, \
                 tc.tile_pool(name="c_sb", bufs=3) as csb:
                def h2_out(nt, wm, hsb, pools):
                    tpp, esb = pools
                    n0 = nt * P
                    tp = tpp.tile([P, P], F32, tag="tp", space="PSUM")
                    nc.tensor.transpose(out=tp[:, :wm], in_=hsb[:wm, 0:D1],
                                        identity=ident[:wm, :wm])
                    h2t = esb.tile([P, P], F32, tag="ht")
                    nc.vector.tensor_copy(out=h2t[:, :wm], in_=tp[:, :wm])
                    fps = cfp.tile([P, NCLS], F32, tag="fps", space="PSUM")
                    nc.tensor.matmul(out=fps[:wm, :], lhsT=h2t[:, :wm],
                                     rhs=pja_t[:], start=True, stop=False)
                    nft = csb.tile([FM, P], F32, tag="nft")
                    nc.sync.dma_start(out=nft[:, :wm], in_=NFMT[:, n0:n0 + wm])
                    nc.tensor.matmul(out=fps[:wm, :], lhsT=nft[:, :wm],
                                     rhs=pjb_t[:], start=False, stop=False)
                    nc.tensor.matmul(out=fps[:wm, :], lhsT=ones_row[:1, :wm],
                                     rhs=pbias_t[:], start=False, stop=True)
                    ot = csb.tile([P, NCLS], F32, tag="ot")
                    nc.vector.tensor_copy(out=ot[:wm, :], in_=fps[:wm, :])
                    nc.sync.dma_start(out=out_d[n0:n0 + wm, :], in_=ot[:wm, :])

                if stage >= 6:
                 edge_phase(T1F, T1S, C1, D1, h2_out)

    nc.finalize()
    return nc


_CACHE = {}


def _get_program(cfg_key, cfg):
    if cfg_key not in _CACHE:
        _CACHE[cfg_key] = _build(cfg)
    return _CACHE[cfg_key]


def kernel(**inputs) -> np.ndarray:
    cfg, in_maps = _prep(inputs)
    cfg_key = (cfg["N"], cfg["E"], cfg["Din"], cfg["D0"], cfg["D1"],
               cfg["FM"], cfg["NCLS"], tuple(cfg["TPT"]),
               tuple(tuple(r) for r in cfg["SZ"]))
    nc = _get_program(cfg_key, cfg)
    res = run_bass_kernel_spmd(nc, in_maps, list(range(cfg["n_cores"])))
    out = np.concatenate(
        [res.results[c]["out"] for c in range(cfg["n_cores"])], axis=0)
    return out.astype(np.float32)
